# revision 58
# baseline (speedup 1.0000x reference)
"""Trainium2 Bass kernel for nn_AttentionSeqToMasked (dense transformer attention).

Full-input contract: kernel(**inputs) takes the unsharded numpy inputs and
returns the full [B, SQ, H*D_V] float32 output.

Sharding (8 cores): data parallel over batch (B=4 -> 2 cores per batch) x
tensor parallel over heads (16 heads -> 8 per core). Each core computes
attention for one (batch, head-half) pair; host gathers the slices.

v2 design (vs the 403us baseline):
  - Startup: x/w DMAs are issued in priority-chunked order (wq, wk, then the
    qc=0 chunks of xk/xq first) so the first scores tile unblocks ~15us in
    instead of ~53us.
  - exp split: kt tiles 0..13 on ScalarE (exact exp, bf16 out); kt 14..15 on
    VectorE via a blended-Schraudolph bit trick (two half-octave-offset bf16
    reads of one magic-rounded fp32 word, summed; max rel err ~1.4%).
    A constant SHIFT=1.5 is folded into both paths' biases (cancels in the
    softmax ratio; keeps the two paths on one denominator scale).
  - Epilogue: AV psum blocks [65, 512] (64 numerator rows + ones-row
    denominator) are copied to SBUF and DMA'd out raw; the host does the
    divide + [d_v, q] -> [q, d_v] transpose. Removes all PE transposes,
    reciprocals and staging multiplies from the device.
  - Everything stays bf16 (fp8 fails accuracy: per-element exp/v quantization
    error passes straight through to the output).

Scheduling: projection work for later pairs is chopped into ~0.85us psum-chunk
halves and interleaved into the attention stream as TensorE filler.
"""

import os
from contextlib import ExitStack

import numpy as np
import ml_dtypes

import concourse.bass as bass
import concourse.bacc as bacc
import concourse.mybir as mybir
import concourse.tile as tile
from concourse.bass_utils import run_bass_kernel_spmd

# Problem shape (hardcoded per contract)
B, SQ, SK = 4, 2048, 2048
D_PRE = 1024
H, D_QK, D_V = 16, 64, 64
N_CORES = 8
HALF = (H // 2) * D_QK  # 512 columns of the projection handled per core
N_PAIRS = 4  # head pairs per core
S_CHUNK = 512  # moving free-dim per matmul
N_DT = D_PRE // 128  # d_pre tiles of 128
N_KT = SK // 128  # key tiles of 128
N_QC = SQ // S_CHUNK  # query chunks of 512
MASK_NEG = -30000.0

# exp handling: exp(l - SHIFT) everywhere (cancels in softmax).
SHIFT = 1.5
L2E = 1.4426950408889634
MAGIC = 12582912.0  # 1.5 * 2^23: fp32 add forces RNE of the fraction
# DVE fast-exp: u = sc*A + Bc (+MAGIC); ex = bf16view(u) + bf16view(u+64)
FE_A = 0.125 * L2E * 128.0
FE_A1 = -169.5  # blend bias (tuned: max rel err 1.43%)
FE_B = (-SHIFT * L2E + 127.0) * 128.0 + FE_A1
DVE_KTS = (14, 15)  # kt tiles whose exp runs on VectorE
N_ROWS_OUT = N_PAIRS * 2 * 65  # 520 rows: [pair][head][64 num + 1 den]

F32 = mybir.dt.float32
BF16 = mybir.dt.bfloat16
BF16_NP = np.dtype(ml_dtypes.bfloat16)

_COMPILED = None


def _build_program():
    nc = bacc.Bacc("TRN2", target_bir_lowering=False, debug=False)

    # DRAM I/O (names are the in_map keys). x tensors are chunk-blocked on
    # the host ([qc][d_pre][512] dense blocks) so each chunk DMA is a fully
    # contiguous 128KB read (1KB-burst strided reads halve DMA bandwidth).
    xq = nc.dram_tensor("xq", [N_QC, D_PRE, S_CHUNK], BF16, kind="ExternalInput").ap()
    xk = nc.dram_tensor("xk", [N_QC, D_PRE, S_CHUNK], BF16, kind="ExternalInput").ap()
    xv = nc.dram_tensor("xv", [N_QC, D_PRE, S_CHUNK], BF16, kind="ExternalInput").ap()
    wq = nc.dram_tensor("wq", [D_PRE, HALF], BF16, kind="ExternalInput").ap()
    wk = nc.dram_tensor("wk", [D_PRE, HALF], BF16, kind="ExternalInput").ap()
    # v weights with a zero column appended per head (ones column generator)
    wv = nc.dram_tensor("wv", [D_PRE, N_PAIRS * 130], BF16, kind="ExternalInput").ap()
    bq = nc.dram_tensor("bq", [128, N_PAIRS], F32, kind="ExternalInput").ap()
    bk = nc.dram_tensor("bk", [128, N_PAIRS], F32, kind="ExternalInput").ap()
    bv = nc.dram_tensor("bv", [128, N_PAIRS * 130], F32, kind="ExternalInput").ap()
    mb = nc.dram_tensor("mb", [128, N_KT], F32, kind="ExternalInput").ap()
    dvea = nc.dram_tensor("dvea", [128, N_KT], F32, kind="ExternalInput").ap()
    dveb = nc.dram_tensor("dveb", [128, N_KT], F32, kind="ExternalInput").ap()
    out = nc.dram_tensor("out", [N_ROWS_OUT, SQ], F32, kind="ExternalOutput").ap()

    with tile.TileContext(nc) as tc:
        _emit(tc, xq, xk, xv, wq, wk, wv, bq, bk, bv, mb, dvea, dveb, out)

    nc.compile()
    return nc


def _emit(tc, xq, xk, xv, wq, wk, wv, bq, bk, bv, mb, dvea, dveb, out):
    nc = tc.nc

    with ExitStack() as ctx:
        # ---- pools ----
        xp = ctx.enter_context(tc.tile_pool(name="x", bufs=3))
        wp = ctx.enter_context(tc.tile_pool(name="w", bufs=1))
        cp = ctx.enter_context(tc.tile_pool(name="const", bufs=1))
        qkvp = ctx.enter_context(tc.tile_pool(name="qkv", bufs=1))
        expp = ctx.enter_context(tc.tile_pool(name="exp", bufs=3))
        fep = ctx.enter_context(tc.tile_pool(name="fe", bufs=1))
        stgp = ctx.enter_context(tc.tile_pool(name="stg", bufs=2))

        proj_ps = ctx.enter_context(tc.tile_pool(name="proj_ps", bufs=1, space="PSUM"))
        sc_ps = ctx.enter_context(tc.tile_pool(name="sc_ps", bufs=2, space="PSUM"))
        av_ps = ctx.enter_context(tc.tile_pool(name="av_ps", bufs=2, space="PSUM"))

        # ---- PE warmup: the HAM clock gate keeps the PE at 1.2 GHz until
        # ~3.4us of sustained activity. Burn accumulating dummy matmuls on a
        # zeroed tile during the input-DMA wait (one psum tile, start=False
        # chain: back-to-back, no buffer-recycle serialization).
        warm = cp.tile([128, 512], BF16, name="warm")
        nc.vector.memset(warm, 0.0)
        ones_sb = cp.tile([128, 1], BF16, name="ones_sb")
        nc.vector.memset(ones_sb, 1.0)
        wps = proj_ps.tile([128, S_CHUNK], F32, name="warmps", tag="proj")
        for wi in range(24):
            nc.tensor.matmul(
                wps, lhsT=warm[:, 0:128], rhs=warm, start=(wi == 0), stop=(wi == 23)
            )

        # ---- constants ----
        mb_sb = cp.tile([128, N_KT], F32, name="mb_sb")
        nc.sync.dma_start(mb_sb, mb)
        dvea_sb = cp.tile([128, N_KT], F32, name="dvea_sb")
        nc.sync.dma_start(dvea_sb, dvea)
        dveb_sb = cp.tile([128, N_KT], F32, name="dveb_sb")
        nc.sync.dma_start(dveb_sb, dveb)
        bq_sb = cp.tile([128, N_PAIRS], F32, name="bq_sb")
        nc.sync.dma_start(bq_sb, bq)
        bk_sb = cp.tile([128, N_PAIRS], F32, name="bk_sb")
        nc.sync.dma_start(bk_sb, bk)
        bv_sb = cp.tile([128, N_PAIRS * 130], F32, name="bv_sb")
        nc.sync.dma_start(bv_sb, bv)

        # ---- streamed loads, priority-chunked: the first scores tile needs
        # only (wq, wk, xk chunk0, xq chunk0); the kt sweep then consumes the
        # rest of xk before more of xq is needed; v comes last. ----
        # Priority order: critical path to the first exp is wq+wk+xk0+xq0;
        # the kt sweep then needs the rest of xk at ~1.3us/kt; v (wv+xv)
        # must land before the deferred qc0 AV flush; the rest of xq is
        # needed one qc-block (~18us) at a time. Each load is ONE 1MB DMA
        # (dispatch on the sync engine costs ~650ns per DMA instruction, so
        # per-dt-tile DMAs would serialize on dispatch).
        def alloc_x(pfx):
            big = xp.tile([128, N_DT * SQ], BF16, name=f"{pfx}big", tag="x")
            return big, [big[:, i * SQ : (i + 1) * SQ] for i in range(N_DT)]

        def load_x_chunk(big, xap, c, eng=None):
            dst = big.rearrange("p (dt s) -> p dt s", dt=N_DT)[
                :, :, c * S_CHUNK : (c + 1) * S_CHUNK
            ]
            src = xap[c].rearrange("(dt p) s -> p dt s", p=128)
            (eng or nc.sync).dma_start(dst, src)

        def load_w(wap, pfx, width, eng=None):
            big = wp.tile([128, N_DT * width], BF16, name=f"{pfx}big", tag=pfx)
            (eng or nc.sync).dma_start(
                big.rearrange("p (dt w) -> p dt w", dt=N_DT),
                wap.rearrange("(dt p) w -> p dt w", p=128),
            )
            return [big[:, i * width : (i + 1) * width] for i in range(N_DT)]

        # All input DMAs go on ONE queue in strict priority order: HBM
        # bandwidth (~300 GB/s/core) is shared, so parallel queues just
        # steal bandwidth from the critical path. Land order = need order:
        # first exp needs wq+wk+xk0+xq0 (~18us); the kt sweep eats xk at
        # ~1.1us/kt; v (wv+xv chunks) must land by the qc0 AV flush (~40us);
        # xq chunks are needed one qc-block (~18us) apart.
        xq_big, xq_sb = alloc_x("xq")
        xk_big, xk_sb = alloc_x("xk")
        xv_big, xv_sb = alloc_x("xv")
        wq_sb = load_w(wq, "wq", HALF)
        wk_sb = load_w(wk, "wk", HALF)
        load_x_chunk(xk_big, xk, 0)
        load_x_chunk(xq_big, xq, 0)
        wv_sb = load_w(wv, "wv", N_PAIRS * 130)
        load_x_chunk(xv_big, xv, 0)
        load_x_chunk(xk_big, xk, 1)
        load_x_chunk(xq_big, xq, 1)
        load_x_chunk(xk_big, xk, 2)
        load_x_chunk(xk_big, xk, 3)
        for c in range(1, N_QC):
            load_x_chunk(xv_big, xv, c)
        load_x_chunk(xq_big, xq, 2)
        load_x_chunk(xq_big, xq, 3)

        v_tiles = {}  # (pair, kt) -> [128, 130] bf16 tile
        qkT = {}  # (pfx, pair) -> [128, SQ] bf16 tile

        def qk_tile(pfx, pair):
            if (pfx, pair) not in qkT:
                qkT[(pfx, pair)] = qkvp.tile(
                    [128, SQ], BF16, name=f"{pfx}T{pair}", tag=f"{pfx}T", bufs=2
                )
            return qkT[(pfx, pair)]

        proj_ps_open = {}

        def emit_qk_chunk(pair, pfx, qc, half=None):
            # one [128, 512] projection chunk: 8 accumulating MMs + bias copy.
            # half=0/1 emits only the first/second 4 contraction MMs (filler
            # granularity); half=None emits the whole chunk.
            dst = qk_tile(pfx, pair)
            w_sb = wq_sb if pfx == "q" else wk_sb
            b_sb = bq_sb if pfx == "q" else bk_sb
            x_sb = xq_sb if pfx == "q" else xk_sb
            key = (pair, pfx, qc)
            if half == 1:
                ps = proj_ps_open.pop(key)
            else:
                ps = proj_ps.tile(
                    [128, S_CHUNK], F32, name=f"{pfx}ps{pair}_{qc}", tag="proj"
                )
            dts = range(N_DT) if half is None else range(half * 4, half * 4 + 4)
            for dt_i in dts:
                nc.tensor.matmul(
                    ps,
                    lhsT=w_sb[dt_i][:, pair * 128 : (pair + 1) * 128],
                    rhs=x_sb[dt_i][:, qc * S_CHUNK : (qc + 1) * S_CHUNK],
                    start=(dt_i == 0),
                    stop=(dt_i == N_DT - 1),
                )
            if half == 0:
                proj_ps_open[key] = ps
            else:
                nc.vector.tensor_scalar_add(
                    dst[:, qc * S_CHUNK : (qc + 1) * S_CHUNK],
                    ps,
                    b_sb[:, pair : pair + 1],
                )

        def emit_v_chunk(g, st):
            # v projection for pairs (2g, 2g+1), one key tile: N=260 matmuls
            ps = proj_ps.tile([128, S_CHUNK], F32, name=f"vps{g}_{st}", tag="proj")
            for dt_i in range(N_DT):
                nc.tensor.matmul(
                    ps[:, 0:260],
                    lhsT=xv_sb[dt_i][:, st * 128 : (st + 1) * 128],
                    rhs=wv_sb[dt_i][:, g * 260 : (g + 1) * 260],
                    start=(dt_i == 0),
                    stop=(dt_i == N_DT - 1),
                )
            for j in range(2):
                pair = 2 * g + j
                vt = qkvp.tile(
                    [128, 130], BF16, name=f"v{pair}_{st}", tag="v", bufs=3 * N_KT
                )
                nc.vector.tensor_add(
                    vt,
                    ps[:, j * 130 : (j + 1) * 130],
                    bv_sb[:, pair * 130 : (pair + 1) * 130],
                )
                v_tiles[(pair, st)] = vt

        # filler queue: projection chunk units (~0.85us of PE each), popped as
        # TensorE filler inside the attention stream. qk chunks are split into
        # 4-MM halves to match the per-slot budget. Ordering constraints:
        # qk(p) before pair-p attention, vg1 before pair-2 attention.
        filler = []

        def _qk_half(pair, pfx, qc, half):
            return lambda: emit_qk_chunk(pair, pfx, qc, half)

        for pfx in ("q", "k"):
            for c in range(N_QC):
                filler.append(_qk_half(1, pfx, c, 0))
                filler.append(_qk_half(1, pfx, c, 1))
        filler += [(lambda st=st: emit_v_chunk(1, st)) for st in range(N_KT)]
        for pfx in ("q", "k"):
            for c in range(N_QC):
                filler.append(_qk_half(2, pfx, c, 0))
                filler.append(_qk_half(2, pfx, c, 1))
        for pfx in ("q", "k"):
            for c in range(N_QC):
                filler.append(_qk_half(3, pfx, c, 0))
                filler.append(_qk_half(3, pfx, c, 1))

        def pop_filler():
            if filler:
                filler.pop(0)()

        # prologue: pair-0 projections, ordered so the first scores tile
        # (needing only the qc=0 chunks of qT0/kT0) unblocks ASAP
        emit_qk_chunk(0, "k", 0)
        emit_qk_chunk(0, "q", 0)
        for c in range(1, N_QC):
            emit_qk_chunk(0, "k", c)
        for c in range(1, N_QC):
            emit_qk_chunk(0, "q", c)
        for st in range(N_KT):
            emit_v_chunk(0, st)

        # ---- software-pipelined attention stream over (pair, qc, kt) ----
        iters = [
            (pair, qc, kt)
            for pair in range(N_PAIRS)
            for qc in range(N_QC)
            for kt in range(N_KT)
        ]
        sc_map = {}
        av_map = {}

        def emit_scores(i):
            pair, qc, kt = iters[i]
            qT = qk_tile("q", pair)
            kT = qk_tile("k", pair)
            sc = sc_ps.tile([128, 1024], F32, name=f"sc{pair}_{qc}_{kt}", tag="sc")
            # scoresT for heads A and B, packed in PE row groups
            nc.tensor.matmul(
                sc[:, 0:512],
                lhsT=kT[0:64, kt * 128 : (kt + 1) * 128],
                rhs=qT[0:64, qc * S_CHUNK : (qc + 1) * S_CHUNK],
                start=True,
                stop=True,
            )
            nc.tensor.matmul(
                sc[:, 512:1024],
                lhsT=kT[64:128, kt * 128 : (kt + 1) * 128],
                rhs=qT[64:128, qc * S_CHUNK : (qc + 1) * S_CHUNK],
                start=True,
                stop=True,
            )
            sc_map[i] = sc

        def emit_exp(i):
            # returns the bf16 [128, 1024] exp tile for iteration i
            pair, qc, kt = iters[i]
            sc = sc_map.pop(i)
            if kt in DVE_KTS:
                u = fep.tile([128, 1024], F32, name=f"u{pair}_{qc}_{kt}", tag="u", bufs=2)
                nc.vector.tensor_scalar(
                    u,
                    sc,
                    dvea_sb[:, kt : kt + 1],
                    dveb_sb[:, kt : kt + 1],
                    mybir.AluOpType.mult,
                    mybir.AluOpType.add,
                )
                u2 = fep.tile([128, 1024], F32, name=f"u2{pair}_{qc}_{kt}", tag="u2", bufs=1)
                nc.vector.tensor_scalar_add(u2, u, 64.0)
                ex = fep.tile([128, 1024], BF16, name=f"fx{pair}_{qc}_{kt}", tag="fx", bufs=3)
                z1 = u.bitcast(BF16).rearrange("p (n two) -> p n two", two=2)[:, :, 0]
                z2 = u2.bitcast(BF16).rearrange("p (n two) -> p n two", two=2)[:, :, 0]
                nc.vector.tensor_add(ex, z1, z2)
            else:
                ex = expp.tile(
                    [128, 1024], BF16, name=f"ex{pair}_{qc}_{kt}", tag="ex", bufs=14
                )
                nc.scalar.activation(
                    ex,
                    sc,
                    mybir.ActivationFunctionType.Exp,
                    bias=mb_sb[:, kt : kt + 1],
                    scale=0.125,
                )
            return ex

        def emit_epilogue(pair, qc, av_a, av_b):
            # [65, 512] numerator+den blocks -> SBUF -> DRAM (host normalizes)
            stg = stgp.tile([65, 1024], F32, name=f"st{pair}_{qc}", tag="stg")
            nc.vector.tensor_copy(stg[:, 0:512], av_a)
            nc.vector.tensor_copy(stg[:, 512:1024], av_b)
            for h_i in range(2):
                nc.sync.dma_start(
                    out[
                        pair * 130 + h_i * 65 : pair * 130 + h_i * 65 + 65,
                        qc * S_CHUNK : (qc + 1) * S_CHUNK,
                    ],
                    stg[:, h_i * 512 : (h_i + 1) * 512],
                )

        def emit_av(pair, qc, kt, ex):
            if kt == 0:
                av_map[(pair, qc)] = (
                    av_ps.tile([65, S_CHUNK], F32, name=f"ava{pair}_{qc}", tag="av"),
                    av_ps.tile([65, S_CHUNK], F32, name=f"avb{pair}_{qc}", tag="av"),
                )
            av_a, av_b = av_map[(pair, qc)]
            nc.tensor.matmul(
                av_a,
                lhsT=v_tiles[(pair, kt)][:, 0:65],
                rhs=ex[:, 0:512],
                start=(kt == 0),
                stop=(kt == N_KT - 1),
            )
            nc.tensor.matmul(
                av_b,
                lhsT=v_tiles[(pair, kt)][:, 65:130],
                rhs=ex[:, 512:1024],
                start=(kt == 0),
                stop=(kt == N_KT - 1),
            )

        # Emission in 2-iteration blocks, software-pipelined:
        #   block b: exps (2b, 2b+1) | AV burst (2b-2, 2b-1) | scores (2b+2,
        #   2b+3) | one filler unit. The AV inputs are always two blocks old,
        #   so the 4-matmul AV burst never waits mid-stream; batching halves
        #   the PE stream-switch tax. Iters 0..15 (pair 0, qc 0) defer their
        #   AVs entirely so the exp engines start while the v projection still
        #   waits on the xv DMA.
        emit_scores(0)
        emit_scores(1)
        ex_map = {}
        n_it = len(iters)

        def emit_av_i(i):
            pair, qc, kt = iters[i]
            emit_av(pair, qc, kt, ex_map.pop(i))
            if kt == N_KT - 1:
                emit_epilogue(pair, qc, *av_map.pop((pair, qc)))

        next_av = 0
        for b in range(n_it // 2):
            i0, i1 = 2 * b, 2 * b + 1
            for i in (i0, i1):
                ex_map[i] = emit_exp(i)
            if b >= 8:
                # warmup: (pair 0, qc 0) AVs were deferred while xv was in
                # flight; catch up at <=6 AVs per block (8 near the end to
                # drain the tail) to keep the PE queue from blocking on
                # not-yet-projected v tiles.
                target = 2 * b - 2
                cap = 8 if b >= n_it // 2 - 4 else 6
                n_emit = min(cap, target - next_av)
                for _ in range(n_emit):
                    emit_av_i(next_av)
                    next_av += 1
            if i1 + 2 < n_it:
                emit_scores(i1 + 1)
                emit_scores(i1 + 2)
            # spread the projection filler evenly (3 of every 4 blocks)
            # instead of front-loading it into pairs 0-1
            if b % 4 != 3:
                pop_filler()
        # drain the remaining AVs + epilogues
        while next_av < n_it:
            emit_av_i(next_av)
            next_av += 1

        assert not filler, f"{len(filler)} filler chunks left unscheduled"
        assert not ex_map and not av_map and not sc_map


def _prep_core_inputs(pre_qs, pre_ks, pre_vs, k_mask, q_w, q_b, k_w, k_b, v_w, v_b, core):
    b = core // 2
    hh = core % 2
    cols = slice(HALF * hh, HALF * (hh + 1))

    def chunk_blocked(x):
        # [S, D_PRE] -> [N_QC, D_PRE, S_CHUNK] contiguous blocks of x^T
        xt = x.T.astype(BF16_NP)  # [D_PRE, S]
        return np.ascontiguousarray(
            xt.reshape(D_PRE, N_QC, S_CHUNK).transpose(1, 0, 2)
        )

    xq = chunk_blocked(pre_qs[b])
    xk = chunk_blocked(pre_ks[b])
    xv = chunk_blocked(pre_vs[b])
    wq = np.ascontiguousarray(q_w[:, cols]).astype(BF16_NP)
    wk = np.ascontiguousarray(k_w[:, cols]).astype(BF16_NP)

    wv_core = v_w[:, cols].astype(np.float32)
    wv = np.zeros((D_PRE, N_PAIRS * 130), dtype=np.float32)
    bv_core = v_b[cols].astype(np.float32)
    bv_ext = np.zeros(N_PAIRS * 130, dtype=np.float32)
    for p in range(N_PAIRS):
        wv[:, p * 130 : p * 130 + 64] = wv_core[:, p * 128 : p * 128 + 64]
        wv[:, p * 130 + 65 : p * 130 + 129] = wv_core[:, p * 128 + 64 : p * 128 + 128]
        bv_ext[p * 130 : p * 130 + 64] = bv_core[p * 128 : p * 128 + 64]
        bv_ext[p * 130 + 64] = 1.0
        bv_ext[p * 130 + 65 : p * 130 + 129] = bv_core[p * 128 + 64 : p * 128 + 128]
        bv_ext[p * 130 + 129] = 1.0
    wv = wv.astype(BF16_NP)

    bq = np.ascontiguousarray(q_b[cols].astype(np.float32).reshape(N_PAIRS, 128).T)
    bk = np.ascontiguousarray(k_b[cols].astype(np.float32).reshape(N_PAIRS, 128).T)
    bv_full = np.ascontiguousarray(np.tile(bv_ext[None, :], (128, 1)))

    # ScalarE path: mask True -> -SHIFT, False -> MASK_NEG (exp underflows to 0)
    mrow = np.where(k_mask[b], -SHIFT, MASK_NEG).astype(np.float32)
    mb = np.ascontiguousarray(mrow.reshape(N_KT, 128).T)
    # VectorE fast-exp path: masked-out keys get A=0, B=MAGIC (-> bf16 bits 0)
    arow = np.where(k_mask[b], FE_A, 0.0).astype(np.float32)
    brow = np.where(k_mask[b], FE_B + MAGIC, MAGIC).astype(np.float32)
    dvea = np.ascontiguousarray(arow.reshape(N_KT, 128).T)
    dveb = np.ascontiguousarray(brow.reshape(N_KT, 128).T)

    return {
        "xq": xq,
        "xk": xk,
        "xv": xv,
        "wq": wq,
        "wk": wk,
        "wv": wv,
        "bq": bq,
        "bk": bk,
        "bv": bv_full,
        "mb": mb,
        "dvea": dvea,
        "dveb": dveb,
    }


def kernel(pre_qs, pre_ks, pre_vs, k_mask, q_w, q_b, k_w, k_b, v_w, v_b):
    global _COMPILED
    args = (pre_qs, pre_ks, pre_vs, k_mask, q_w, q_b, k_w, k_b, v_w, v_b)
    args = tuple(np.asarray(a) for a in args)

    if _COMPILED is None:
        _COMPILED = _build_program()
    nc = _COMPILED

    in_maps = [_prep_core_inputs(*args, core=c) for c in range(N_CORES)]

    trace = bool(int(os.environ.get("BASS_KERNEL_TRACE", "0")))
    res = run_bass_kernel_spmd(
        nc,
        in_maps,
        core_ids=list(range(N_CORES)),
        trace=trace,
    )
    if trace:
        kernel.last_results = res

    out = np.empty((B, SQ, H * D_V), dtype=np.float32)
    for c in range(N_CORES):
        b = c // 2
        hh = c % 2
        r = res.results[c]["out"]  # [520, 2048] fp32
        for p in range(N_PAIRS):
            for h_i in range(2):
                blk = r[p * 130 + h_i * 65 : p * 130 + h_i * 65 + 65]
                num = blk[0:64]
                den = blk[64]
                head = hh * 8 + 2 * p + h_i
                out[b, :, head * 64 : (head + 1) * 64] = (num / den).T
    return out


# revision 62
# speedup vs baseline: 1.0508x; 1.0508x over previous
"""Trainium2 Bass kernel for nn_AttentionSeqToMasked (dense transformer attention).

Full-input contract: kernel(**inputs) takes the unsharded numpy inputs and
returns the full [B, SQ, H*D_V] float32 output.

Sharding (8 cores): data parallel over batch (B=4 -> 2 cores per batch) x
tensor parallel over heads (16 heads -> 8 per core). Each core computes
attention for one (batch, head-half) pair; host gathers the slices.

v2 design (vs the 403us baseline):
  - Startup: x/w DMAs are issued in priority-chunked order (wq, wk, then the
    qc=0 chunks of xk/xq first) so the first scores tile unblocks ~15us in
    instead of ~53us.
  - exp split: kt tiles 0..13 on ScalarE (exact exp, bf16 out); kt 14..15 on
    VectorE via a blended-Schraudolph bit trick (two half-octave-offset bf16
    reads of one magic-rounded fp32 word, summed; max rel err ~1.4%).
    A constant SHIFT=1.5 is folded into both paths' biases (cancels in the
    softmax ratio; keeps the two paths on one denominator scale).
  - Epilogue: AV psum blocks [65, 512] (64 numerator rows + ones-row
    denominator) are copied to SBUF and DMA'd out raw; the host does the
    divide + [d_v, q] -> [q, d_v] transpose. Removes all PE transposes,
    reciprocals and staging multiplies from the device.
  - Everything stays bf16 (fp8 fails accuracy: per-element exp/v quantization
    error passes straight through to the output).

Scheduling: projection work for later pairs is chopped into ~0.85us psum-chunk
halves and interleaved into the attention stream as TensorE filler.
"""

import os
from contextlib import ExitStack

import numpy as np
import ml_dtypes

import concourse.bass as bass
import concourse.bacc as bacc
import concourse.mybir as mybir
import concourse.tile as tile
from concourse.bass_utils import run_bass_kernel_spmd

# Problem shape (hardcoded per contract)
B, SQ, SK = 4, 2048, 2048
D_PRE = 1024
H, D_QK, D_V = 16, 64, 64
N_CORES = 8
HALF = (H // 2) * D_QK  # 512 columns of the projection handled per core
N_PAIRS = 4  # head pairs per core
S_CHUNK = 512  # moving free-dim per matmul
N_DT = D_PRE // 128  # d_pre tiles of 128
N_KT = SK // 128  # key tiles of 128
N_QC = SQ // S_CHUNK  # query chunks of 512
MASK_NEG = -30000.0

# exp handling: exp(l - SHIFT) everywhere (cancels in softmax).
SHIFT = 1.5
L2E = 1.4426950408889634
MAGIC = 12582912.0  # 1.5 * 2^23: fp32 add forces RNE of the fraction
# DVE fast-exp: u = sc*A + Bc (+MAGIC); ex = bf16view(u) + bf16view(u+64)
FE_A = 0.125 * L2E * 128.0
FE_A1 = -169.5  # blend bias (tuned: max rel err 1.43%)
FE_B = (-SHIFT * L2E + 127.0) * 128.0 + FE_A1
DVE_KTS = ()  # kt tiles whose exp runs on VectorE
N_ROWS_OUT = N_PAIRS * 2 * 65  # 520 rows: [pair][head][64 num + 1 den]

F32 = mybir.dt.float32
BF16 = mybir.dt.bfloat16
BF16_NP = np.dtype(ml_dtypes.bfloat16)

_COMPILED = None


def _build_program():
    nc = bacc.Bacc("TRN2", target_bir_lowering=False, debug=False)

    # DRAM I/O (names are the in_map keys). x tensors are chunk-blocked on
    # the host ([qc][d_pre][512] dense blocks) so each chunk DMA is a fully
    # contiguous 128KB read (1KB-burst strided reads halve DMA bandwidth).
    xq = nc.dram_tensor("xq", [N_QC, D_PRE, S_CHUNK], BF16, kind="ExternalInput").ap()
    xk = nc.dram_tensor("xk", [N_QC, D_PRE, S_CHUNK], BF16, kind="ExternalInput").ap()
    xv = nc.dram_tensor("xv", [N_QC, D_PRE, S_CHUNK], BF16, kind="ExternalInput").ap()
    wq = nc.dram_tensor("wq", [D_PRE, HALF], BF16, kind="ExternalInput").ap()
    wk = nc.dram_tensor("wk", [D_PRE, HALF], BF16, kind="ExternalInput").ap()
    # v weights with a zero column appended per head (ones column generator)
    wv = nc.dram_tensor("wv", [D_PRE, N_PAIRS * 130], BF16, kind="ExternalInput").ap()
    bq = nc.dram_tensor("bq", [128, N_PAIRS], F32, kind="ExternalInput").ap()
    bk = nc.dram_tensor("bk", [128, N_PAIRS], F32, kind="ExternalInput").ap()
    bv = nc.dram_tensor("bv", [128, N_PAIRS * 130], F32, kind="ExternalInput").ap()
    mb = nc.dram_tensor("mb", [128, N_KT], F32, kind="ExternalInput").ap()
    dvea = nc.dram_tensor("dvea", [128, N_KT], F32, kind="ExternalInput").ap()
    dveb = nc.dram_tensor("dveb", [128, N_KT], F32, kind="ExternalInput").ap()
    out = nc.dram_tensor("out", [N_ROWS_OUT, SQ], F32, kind="ExternalOutput").ap()

    with tile.TileContext(nc) as tc:
        _emit(tc, xq, xk, xv, wq, wk, wv, bq, bk, bv, mb, dvea, dveb, out)

    nc.compile()
    return nc


def _emit(tc, xq, xk, xv, wq, wk, wv, bq, bk, bv, mb, dvea, dveb, out):
    nc = tc.nc

    with ExitStack() as ctx:
        # ---- pools ----
        xp = ctx.enter_context(tc.tile_pool(name="x", bufs=3))
        wp = ctx.enter_context(tc.tile_pool(name="w", bufs=1))
        cp = ctx.enter_context(tc.tile_pool(name="const", bufs=1))
        qkvp = ctx.enter_context(tc.tile_pool(name="qkv", bufs=1))
        expp = ctx.enter_context(tc.tile_pool(name="exp", bufs=3))
        fep = ctx.enter_context(tc.tile_pool(name="fe", bufs=1))
        stgp = ctx.enter_context(tc.tile_pool(name="stg", bufs=2))

        proj_ps = ctx.enter_context(tc.tile_pool(name="proj_ps", bufs=1, space="PSUM"))
        sc_ps = ctx.enter_context(tc.tile_pool(name="sc_ps", bufs=2, space="PSUM"))
        av_ps = ctx.enter_context(tc.tile_pool(name="av_ps", bufs=2, space="PSUM"))

        # ---- PE warmup: the HAM clock gate keeps the PE at 1.2 GHz until
        # ~3.4us of sustained activity. Burn accumulating dummy matmuls on a
        # zeroed tile during the input-DMA wait (one psum tile, start=False
        # chain: back-to-back, no buffer-recycle serialization).
        warm = cp.tile([128, 512], BF16, name="warm")
        nc.vector.memset(warm, 0.0)
        ones_sb = cp.tile([128, 1], BF16, name="ones_sb")
        nc.vector.memset(ones_sb, 1.0)
        wps = proj_ps.tile([128, S_CHUNK], F32, name="warmps", tag="proj")
        for wi in range(24):
            nc.tensor.matmul(
                wps, lhsT=warm[:, 0:128], rhs=warm, start=(wi == 0), stop=(wi == 23)
            )

        # ---- constants ----
        mb_sb = cp.tile([128, N_KT], F32, name="mb_sb")
        nc.sync.dma_start(mb_sb, mb)
        dvea_sb = cp.tile([128, N_KT], F32, name="dvea_sb")
        nc.sync.dma_start(dvea_sb, dvea)
        dveb_sb = cp.tile([128, N_KT], F32, name="dveb_sb")
        nc.sync.dma_start(dveb_sb, dveb)
        bq_sb = cp.tile([128, N_PAIRS], F32, name="bq_sb")
        nc.sync.dma_start(bq_sb, bq)
        bk_sb = cp.tile([128, N_PAIRS], F32, name="bk_sb")
        nc.sync.dma_start(bk_sb, bk)
        bv_sb = cp.tile([128, N_PAIRS * 130], F32, name="bv_sb")
        nc.sync.dma_start(bv_sb, bv)

        # ---- streamed loads, priority-chunked: the first scores tile needs
        # only (wq, wk, xk chunk0, xq chunk0); the kt sweep then consumes the
        # rest of xk before more of xq is needed; v comes last. ----
        # Priority order: critical path to the first exp is wq+wk+xk0+xq0;
        # the kt sweep then needs the rest of xk at ~1.3us/kt; v (wv+xv)
        # must land before the deferred qc0 AV flush; the rest of xq is
        # needed one qc-block (~18us) at a time. Each load is ONE 1MB DMA
        # (dispatch on the sync engine costs ~650ns per DMA instruction, so
        # per-dt-tile DMAs would serialize on dispatch).
        def alloc_x(pfx):
            big = xp.tile([128, N_DT * SQ], BF16, name=f"{pfx}big", tag="x")
            return big, [big[:, i * SQ : (i + 1) * SQ] for i in range(N_DT)]

        def load_x_chunk(big, xap, c, eng=None):
            dst = big.rearrange("p (dt s) -> p dt s", dt=N_DT)[
                :, :, c * S_CHUNK : (c + 1) * S_CHUNK
            ]
            src = xap[c].rearrange("(dt p) s -> p dt s", p=128)
            (eng or nc.sync).dma_start(dst, src)

        def load_w(wap, pfx, width, eng=None):
            big = wp.tile([128, N_DT * width], BF16, name=f"{pfx}big", tag=pfx)
            (eng or nc.sync).dma_start(
                big.rearrange("p (dt w) -> p dt w", dt=N_DT),
                wap.rearrange("(dt p) w -> p dt w", p=128),
            )
            return [big[:, i * width : (i + 1) * width] for i in range(N_DT)]

        # All input DMAs go on ONE queue in strict priority order: HBM
        # bandwidth (~300 GB/s/core) is shared, so parallel queues just
        # steal bandwidth from the critical path. Land order = need order:
        # first exp needs wq+wk+xk0+xq0 (~18us); the kt sweep eats xk at
        # ~1.1us/kt; v (wv+xv chunks) must land by the qc0 AV flush (~40us);
        # xq chunks are needed one qc-block (~18us) apart.
        xq_big, xq_sb = alloc_x("xq")
        xk_big, xk_sb = alloc_x("xk")
        xv_big, xv_sb = alloc_x("xv")
        wq_sb = load_w(wq, "wq", HALF)
        wk_sb = load_w(wk, "wk", HALF)
        load_x_chunk(xk_big, xk, 0)
        load_x_chunk(xq_big, xq, 0)
        wv_sb = load_w(wv, "wv", N_PAIRS * 130)
        load_x_chunk(xv_big, xv, 0)
        load_x_chunk(xk_big, xk, 1)
        load_x_chunk(xq_big, xq, 1)
        load_x_chunk(xk_big, xk, 2)
        load_x_chunk(xk_big, xk, 3)
        for c in range(1, N_QC):
            load_x_chunk(xv_big, xv, c)
        load_x_chunk(xq_big, xq, 2)
        load_x_chunk(xq_big, xq, 3)

        v_tiles = {}  # (pair, kt) -> [128, 130] bf16 tile
        qkT = {}  # (pfx, pair) -> [128, SQ] bf16 tile

        def qk_tile(pfx, pair):
            if (pfx, pair) not in qkT:
                qkT[(pfx, pair)] = qkvp.tile(
                    [128, SQ], BF16, name=f"{pfx}T{pair}", tag=f"{pfx}T", bufs=2
                )
            return qkT[(pfx, pair)]

        proj_ps_open = {}

        def emit_qk_chunk(pair, pfx, qc, half=None):
            # one [128, 512] projection chunk: 8 accumulating MMs + bias copy.
            # half=0/1 emits only the first/second 4 contraction MMs (filler
            # granularity); half=None emits the whole chunk.
            dst = qk_tile(pfx, pair)
            w_sb = wq_sb if pfx == "q" else wk_sb
            b_sb = bq_sb if pfx == "q" else bk_sb
            x_sb = xq_sb if pfx == "q" else xk_sb
            key = (pair, pfx, qc)
            if half == 1:
                ps = proj_ps_open.pop(key)
            else:
                ps = proj_ps.tile(
                    [128, S_CHUNK], F32, name=f"{pfx}ps{pair}_{qc}", tag="proj"
                )
            dts = range(N_DT) if half is None else range(half * 4, half * 4 + 4)
            for dt_i in dts:
                nc.tensor.matmul(
                    ps,
                    lhsT=w_sb[dt_i][:, pair * 128 : (pair + 1) * 128],
                    rhs=x_sb[dt_i][:, qc * S_CHUNK : (qc + 1) * S_CHUNK],
                    start=(dt_i == 0),
                    stop=(dt_i == N_DT - 1),
                )
            if half == 0:
                proj_ps_open[key] = ps
            else:
                nc.vector.tensor_scalar_add(
                    dst[:, qc * S_CHUNK : (qc + 1) * S_CHUNK],
                    ps,
                    b_sb[:, pair : pair + 1],
                )

        def emit_v_chunk(g, st):
            # v projection for pairs (2g, 2g+1), one key tile: N=260 matmuls
            ps = proj_ps.tile([128, S_CHUNK], F32, name=f"vps{g}_{st}", tag="proj")
            for dt_i in range(N_DT):
                nc.tensor.matmul(
                    ps[:, 0:260],
                    lhsT=xv_sb[dt_i][:, st * 128 : (st + 1) * 128],
                    rhs=wv_sb[dt_i][:, g * 260 : (g + 1) * 260],
                    start=(dt_i == 0),
                    stop=(dt_i == N_DT - 1),
                )
            for j in range(2):
                pair = 2 * g + j
                vt = qkvp.tile(
                    [128, 130], BF16, name=f"v{pair}_{st}", tag="v", bufs=3 * N_KT
                )
                nc.vector.tensor_add(
                    vt,
                    ps[:, j * 130 : (j + 1) * 130],
                    bv_sb[:, pair * 130 : (pair + 1) * 130],
                )
                v_tiles[(pair, st)] = vt

        # filler queue: projection chunk units (~0.85us of PE each), popped as
        # TensorE filler inside the attention stream. qk chunks are split into
        # 4-MM halves to match the per-slot budget. Ordering constraints:
        # qk(p) before pair-p attention, vg1 before pair-2 attention.
        filler = []

        def _qk_half(pair, pfx, qc, half):
            return lambda: emit_qk_chunk(pair, pfx, qc, half)

        for pfx in ("q", "k"):
            for c in range(N_QC):
                filler.append(_qk_half(1, pfx, c, 0))
                filler.append(_qk_half(1, pfx, c, 1))
        filler += [(lambda st=st: emit_v_chunk(1, st)) for st in range(N_KT)]
        for pfx in ("q", "k"):
            for c in range(N_QC):
                filler.append(_qk_half(2, pfx, c, 0))
                filler.append(_qk_half(2, pfx, c, 1))
        for pfx in ("q", "k"):
            for c in range(N_QC):
                filler.append(_qk_half(3, pfx, c, 0))
                filler.append(_qk_half(3, pfx, c, 1))

        def pop_filler():
            if filler:
                filler.pop(0)()

        # prologue: pair-0 projections, ordered so the first scores tile
        # (needing only the qc=0 chunks of qT0/kT0) unblocks ASAP
        emit_qk_chunk(0, "k", 0)
        emit_qk_chunk(0, "q", 0)
        for c in range(1, N_QC):
            emit_qk_chunk(0, "k", c)
        for c in range(1, N_QC):
            emit_qk_chunk(0, "q", c)
        for st in range(N_KT):
            emit_v_chunk(0, st)

        # ---- software-pipelined attention stream over (pair, qc, kt) ----
        iters = [
            (pair, qc, kt)
            for pair in range(N_PAIRS)
            for qc in range(N_QC)
            for kt in range(N_KT)
        ]
        sc_map = {}
        av_map = {}

        def emit_scores(i):
            pair, qc, kt = iters[i]
            qT = qk_tile("q", pair)
            kT = qk_tile("k", pair)
            sc = sc_ps.tile([128, 1024], F32, name=f"sc{pair}_{qc}_{kt}", tag="sc")
            # scoresT for heads A and B, packed in PE row groups
            nc.tensor.matmul(
                sc[:, 0:512],
                lhsT=kT[0:64, kt * 128 : (kt + 1) * 128],
                rhs=qT[0:64, qc * S_CHUNK : (qc + 1) * S_CHUNK],
                start=True,
                stop=True,
            )
            nc.tensor.matmul(
                sc[:, 512:1024],
                lhsT=kT[64:128, kt * 128 : (kt + 1) * 128],
                rhs=qT[64:128, qc * S_CHUNK : (qc + 1) * S_CHUNK],
                start=True,
                stop=True,
            )
            sc_map[i] = sc

        def emit_exp(i):
            # returns the bf16 [128, 1024] exp tile for iteration i
            pair, qc, kt = iters[i]
            sc = sc_map.pop(i)
            if kt in DVE_KTS:
                u = fep.tile([128, 1024], F32, name=f"u{pair}_{qc}_{kt}", tag="u", bufs=2)
                nc.vector.tensor_scalar(
                    u,
                    sc,
                    dvea_sb[:, kt : kt + 1],
                    dveb_sb[:, kt : kt + 1],
                    mybir.AluOpType.mult,
                    mybir.AluOpType.add,
                )
                u2 = fep.tile([128, 1024], F32, name=f"u2{pair}_{qc}_{kt}", tag="u2", bufs=1)
                nc.vector.tensor_scalar_add(u2, u, 64.0)
                ex = fep.tile([128, 1024], BF16, name=f"fx{pair}_{qc}_{kt}", tag="fx", bufs=3)
                z1 = u.bitcast(BF16).rearrange("p (n two) -> p n two", two=2)[:, :, 0]
                z2 = u2.bitcast(BF16).rearrange("p (n two) -> p n two", two=2)[:, :, 0]
                nc.vector.tensor_add(ex, z1, z2)
            else:
                ex = expp.tile(
                    [128, 1024], BF16, name=f"ex{pair}_{qc}_{kt}", tag="ex", bufs=20
                )
                nc.scalar.activation(
                    ex,
                    sc,
                    mybir.ActivationFunctionType.Exp,
                    bias=mb_sb[:, kt : kt + 1],
                    scale=0.125,
                )
            return ex

        def emit_epilogue(pair, qc, av_a, av_b):
            # [65, 512] numerator+den blocks -> SBUF -> DRAM (host normalizes)
            stg = stgp.tile([65, 1024], F32, name=f"st{pair}_{qc}", tag="stg")
            nc.vector.tensor_copy(stg[:, 0:512], av_a)
            nc.vector.tensor_copy(stg[:, 512:1024], av_b)
            for h_i in range(2):
                nc.sync.dma_start(
                    out[
                        pair * 130 + h_i * 65 : pair * 130 + h_i * 65 + 65,
                        qc * S_CHUNK : (qc + 1) * S_CHUNK,
                    ],
                    stg[:, h_i * 512 : (h_i + 1) * 512],
                )

        def emit_av(pair, qc, kt, ex):
            if kt == 0:
                av_map[(pair, qc)] = (
                    av_ps.tile([65, S_CHUNK], F32, name=f"ava{pair}_{qc}", tag="av"),
                    av_ps.tile([65, S_CHUNK], F32, name=f"avb{pair}_{qc}", tag="av"),
                )
            av_a, av_b = av_map[(pair, qc)]
            nc.tensor.matmul(
                av_a,
                lhsT=v_tiles[(pair, kt)][:, 0:65],
                rhs=ex[:, 0:512],
                start=(kt == 0),
                stop=(kt == N_KT - 1),
            )
            nc.tensor.matmul(
                av_b,
                lhsT=v_tiles[(pair, kt)][:, 65:130],
                rhs=ex[:, 512:1024],
                start=(kt == 0),
                stop=(kt == N_KT - 1),
            )

        # Emission in 2-iteration blocks, software-pipelined:
        #   block b: exps (2b, 2b+1) | AV burst (2b-2, 2b-1) | scores (2b+2,
        #   2b+3) | one filler unit. The AV inputs are always two blocks old,
        #   so the 4-matmul AV burst never waits mid-stream; batching halves
        #   the PE stream-switch tax. Iters 0..15 (pair 0, qc 0) defer their
        #   AVs entirely so the exp engines start while the v projection still
        #   waits on the xv DMA.
        emit_scores(0)
        emit_scores(1)
        ex_map = {}
        n_it = len(iters)

        def emit_av_i(i):
            pair, qc, kt = iters[i]
            emit_av(pair, qc, kt, ex_map.pop(i))
            if kt == N_KT - 1:
                emit_epilogue(pair, qc, *av_map.pop((pair, qc)))

        next_av = 0
        for b in range(n_it // 2):
            i0, i1 = 2 * b, 2 * b + 1
            for i in (i0, i1):
                ex_map[i] = emit_exp(i)
            if b >= 8:
                # warmup: (pair 0, qc 0) AVs were deferred while xv was in
                # flight; catch up at <=6 AVs per block (8 near the end to
                # drain the tail) to keep the PE queue from blocking on
                # not-yet-projected v tiles.
                target = 2 * b - 2
                cap = 8 if b >= n_it // 2 - 4 else 6
                n_emit = min(cap, target - next_av)
                for _ in range(n_emit):
                    emit_av_i(next_av)
                    next_av += 1
            if i1 + 2 < n_it:
                emit_scores(i1 + 1)
                emit_scores(i1 + 2)
            # spread the projection filler evenly (3 of every 4 blocks)
            # instead of front-loading it into pairs 0-1
            if b % 4 != 3:
                pop_filler()
        # drain the remaining AVs + epilogues
        while next_av < n_it:
            emit_av_i(next_av)
            next_av += 1

        assert not filler, f"{len(filler)} filler chunks left unscheduled"
        assert not ex_map and not av_map and not sc_map


def _prep_core_inputs(pre_qs, pre_ks, pre_vs, k_mask, q_w, q_b, k_w, k_b, v_w, v_b, core):
    b = core // 2
    hh = core % 2
    cols = slice(HALF * hh, HALF * (hh + 1))

    def chunk_blocked(x):
        # [S, D_PRE] -> [N_QC, D_PRE, S_CHUNK] contiguous blocks of x^T
        xt = x.T.astype(BF16_NP)  # [D_PRE, S]
        return np.ascontiguousarray(
            xt.reshape(D_PRE, N_QC, S_CHUNK).transpose(1, 0, 2)
        )

    xq = chunk_blocked(pre_qs[b])
    xk = chunk_blocked(pre_ks[b])
    xv = chunk_blocked(pre_vs[b])
    wq = np.ascontiguousarray(q_w[:, cols]).astype(BF16_NP)
    wk = np.ascontiguousarray(k_w[:, cols]).astype(BF16_NP)

    wv_core = v_w[:, cols].astype(np.float32)
    wv = np.zeros((D_PRE, N_PAIRS * 130), dtype=np.float32)
    bv_core = v_b[cols].astype(np.float32)
    bv_ext = np.zeros(N_PAIRS * 130, dtype=np.float32)
    for p in range(N_PAIRS):
        wv[:, p * 130 : p * 130 + 64] = wv_core[:, p * 128 : p * 128 + 64]
        wv[:, p * 130 + 65 : p * 130 + 129] = wv_core[:, p * 128 + 64 : p * 128 + 128]
        bv_ext[p * 130 : p * 130 + 64] = bv_core[p * 128 : p * 128 + 64]
        bv_ext[p * 130 + 64] = 1.0
        bv_ext[p * 130 + 65 : p * 130 + 129] = bv_core[p * 128 + 64 : p * 128 + 128]
        bv_ext[p * 130 + 129] = 1.0
    wv = wv.astype(BF16_NP)

    bq = np.ascontiguousarray(q_b[cols].astype(np.float32).reshape(N_PAIRS, 128).T)
    bk = np.ascontiguousarray(k_b[cols].astype(np.float32).reshape(N_PAIRS, 128).T)
    bv_full = np.ascontiguousarray(np.tile(bv_ext[None, :], (128, 1)))

    # ScalarE path: mask True -> -SHIFT, False -> MASK_NEG (exp underflows to 0)
    mrow = np.where(k_mask[b], -SHIFT, MASK_NEG).astype(np.float32)
    mb = np.ascontiguousarray(mrow.reshape(N_KT, 128).T)
    # VectorE fast-exp path: masked-out keys get A=0, B=MAGIC (-> bf16 bits 0)
    arow = np.where(k_mask[b], FE_A, 0.0).astype(np.float32)
    brow = np.where(k_mask[b], FE_B + MAGIC, MAGIC).astype(np.float32)
    dvea = np.ascontiguousarray(arow.reshape(N_KT, 128).T)
    dveb = np.ascontiguousarray(brow.reshape(N_KT, 128).T)

    return {
        "xq": xq,
        "xk": xk,
        "xv": xv,
        "wq": wq,
        "wk": wk,
        "wv": wv,
        "bq": bq,
        "bk": bk,
        "bv": bv_full,
        "mb": mb,
        "dvea": dvea,
        "dveb": dveb,
    }


def kernel(pre_qs, pre_ks, pre_vs, k_mask, q_w, q_b, k_w, k_b, v_w, v_b):
    global _COMPILED
    args = (pre_qs, pre_ks, pre_vs, k_mask, q_w, q_b, k_w, k_b, v_w, v_b)
    args = tuple(np.asarray(a) for a in args)

    if _COMPILED is None:
        _COMPILED = _build_program()
    nc = _COMPILED

    in_maps = [_prep_core_inputs(*args, core=c) for c in range(N_CORES)]

    trace = bool(int(os.environ.get("BASS_KERNEL_TRACE", "0")))
    res = run_bass_kernel_spmd(
        nc,
        in_maps,
        core_ids=list(range(N_CORES)),
        trace=trace,
    )
    if trace:
        kernel.last_results = res

    out = np.empty((B, SQ, H * D_V), dtype=np.float32)
    for c in range(N_CORES):
        b = c // 2
        hh = c % 2
        r = res.results[c]["out"]  # [520, 2048] fp32
        for p in range(N_PAIRS):
            for h_i in range(2):
                blk = r[p * 130 + h_i * 65 : p * 130 + h_i * 65 + 65]
                num = blk[0:64]
                den = blk[64]
                head = hh * 8 + 2 * p + h_i
                out[b, :, head * 64 : (head + 1) * 64] = (num / den).T
    return out


# revision 66
# speedup vs baseline: 1.0627x; 1.0113x over previous
"""Trainium2 Bass kernel for nn_AttentionSeqToMasked (dense transformer attention).

Full-input contract: kernel(**inputs) takes the unsharded numpy inputs and
returns the full [B, SQ, H*D_V] float32 output.

Sharding (8 cores): data parallel over batch (B=4 -> 2 cores per batch) x
tensor parallel over heads (16 heads -> 8 per core). Each core computes
attention for one (batch, head-half) pair; host gathers the slices.

v2 design (vs the 403us baseline):
  - Startup: x/w DMAs are issued in priority-chunked order (wq, wk, then the
    qc=0 chunks of xk/xq first) so the first scores tile unblocks ~15us in
    instead of ~53us.
  - exp split: kt tiles 0..13 on ScalarE (exact exp, bf16 out); kt 14..15 on
    VectorE via a blended-Schraudolph bit trick (two half-octave-offset bf16
    reads of one magic-rounded fp32 word, summed; max rel err ~1.4%).
    A constant SHIFT=1.5 is folded into both paths' biases (cancels in the
    softmax ratio; keeps the two paths on one denominator scale).
  - Epilogue: AV psum blocks [65, 512] (64 numerator rows + ones-row
    denominator) are copied to SBUF and DMA'd out raw; the host does the
    divide + [d_v, q] -> [q, d_v] transpose. Removes all PE transposes,
    reciprocals and staging multiplies from the device.
  - Everything stays bf16 (fp8 fails accuracy: per-element exp/v quantization
    error passes straight through to the output).

Scheduling: projection work for later pairs is chopped into ~0.85us psum-chunk
halves and interleaved into the attention stream as TensorE filler.
"""

import os
from contextlib import ExitStack

import numpy as np
import ml_dtypes

import concourse.bass as bass
import concourse.bacc as bacc
import concourse.mybir as mybir
import concourse.tile as tile
from concourse.bass_utils import run_bass_kernel_spmd

# Problem shape (hardcoded per contract)
B, SQ, SK = 4, 2048, 2048
D_PRE = 1024
H, D_QK, D_V = 16, 64, 64
N_CORES = 8
HALF = (H // 2) * D_QK  # 512 columns of the projection handled per core
N_PAIRS = 4  # head pairs per core
S_CHUNK = 512  # moving free-dim per matmul
N_DT = D_PRE // 128  # d_pre tiles of 128
N_KT = SK // 128  # key tiles of 128
N_QC = SQ // S_CHUNK  # query chunks of 512
MASK_NEG = -30000.0

# exp handling: exp(l - SHIFT) everywhere (cancels in softmax).
SHIFT = 1.5
L2E = 1.4426950408889634
MAGIC = 12582912.0  # 1.5 * 2^23: fp32 add forces RNE of the fraction
# DVE fast-exp: u = sc*A + Bc (+MAGIC); ex = bf16view(u) + bf16view(u+64)
FE_A = 0.125 * L2E * 128.0
FE_A1 = -169.5  # blend bias (tuned: max rel err 1.43%)
FE_B = (-SHIFT * L2E + 127.0) * 128.0 + FE_A1
DVE_KTS = ()  # kt tiles whose exp runs on VectorE
N_ROWS_OUT = N_PAIRS * 2 * 65  # 520 rows: [pair][head][64 num + 1 den]

F32 = mybir.dt.float32
BF16 = mybir.dt.bfloat16
BF16_NP = np.dtype(ml_dtypes.bfloat16)

_COMPILED = None


def _build_program():
    nc = bacc.Bacc("TRN2", target_bir_lowering=False, debug=False)

    # DRAM I/O (names are the in_map keys). x tensors are chunk-blocked on
    # the host ([qc][d_pre][512] dense blocks) so each chunk DMA is a fully
    # contiguous 128KB read (1KB-burst strided reads halve DMA bandwidth).
    xq = nc.dram_tensor("xq", [N_QC, D_PRE, S_CHUNK], BF16, kind="ExternalInput").ap()
    xk = nc.dram_tensor("xk", [N_QC, D_PRE, S_CHUNK], BF16, kind="ExternalInput").ap()
    xv = nc.dram_tensor("xv", [N_QC, D_PRE, S_CHUNK], BF16, kind="ExternalInput").ap()
    wq = nc.dram_tensor("wq", [D_PRE, HALF], BF16, kind="ExternalInput").ap()
    wk = nc.dram_tensor("wk", [D_PRE, HALF], BF16, kind="ExternalInput").ap()
    # v weights with a zero column appended per head (ones column generator)
    wv = nc.dram_tensor("wv", [D_PRE, N_PAIRS * 130], BF16, kind="ExternalInput").ap()
    bq = nc.dram_tensor("bq", [128, N_PAIRS], F32, kind="ExternalInput").ap()
    bk = nc.dram_tensor("bk", [128, N_PAIRS], F32, kind="ExternalInput").ap()
    bv = nc.dram_tensor("bv", [128, N_PAIRS * 130], F32, kind="ExternalInput").ap()
    mb = nc.dram_tensor("mb", [128, N_KT], F32, kind="ExternalInput").ap()
    dvea = nc.dram_tensor("dvea", [128, N_KT], F32, kind="ExternalInput").ap()
    dveb = nc.dram_tensor("dveb", [128, N_KT], F32, kind="ExternalInput").ap()
    out = nc.dram_tensor("out", [N_ROWS_OUT, SQ], F32, kind="ExternalOutput").ap()

    with tile.TileContext(nc) as tc:
        _emit(tc, xq, xk, xv, wq, wk, wv, bq, bk, bv, mb, dvea, dveb, out)

    nc.compile()
    return nc


def _emit(tc, xq, xk, xv, wq, wk, wv, bq, bk, bv, mb, dvea, dveb, out):
    nc = tc.nc

    with ExitStack() as ctx:
        # ---- pools ----
        xp = ctx.enter_context(tc.tile_pool(name="x", bufs=3))
        wp = ctx.enter_context(tc.tile_pool(name="w", bufs=1))
        cp = ctx.enter_context(tc.tile_pool(name="const", bufs=1))
        qkvp = ctx.enter_context(tc.tile_pool(name="qkv", bufs=1))
        expp = ctx.enter_context(tc.tile_pool(name="exp", bufs=3))
        fep = ctx.enter_context(tc.tile_pool(name="fe", bufs=1))
        stgp = ctx.enter_context(tc.tile_pool(name="stg", bufs=2))

        proj_ps = ctx.enter_context(tc.tile_pool(name="proj_ps", bufs=1, space="PSUM"))
        sc_ps = ctx.enter_context(tc.tile_pool(name="sc_ps", bufs=2, space="PSUM"))
        av_ps = ctx.enter_context(tc.tile_pool(name="av_ps", bufs=2, space="PSUM"))

        # ---- PE warmup: the HAM clock gate keeps the PE at 1.2 GHz until
        # ~3.4us of sustained activity. Burn accumulating dummy matmuls on a
        # zeroed tile during the input-DMA wait (one psum tile, start=False
        # chain: back-to-back, no buffer-recycle serialization).
        warm = cp.tile([128, 512], BF16, name="warm")
        nc.vector.memset(warm, 0.0)
        ones_sb = cp.tile([128, 1], BF16, name="ones_sb")
        nc.vector.memset(ones_sb, 1.0)
        wps = proj_ps.tile([128, S_CHUNK], F32, name="warmps", tag="proj")
        for wi in range(48):
            nc.tensor.matmul(
                wps, lhsT=warm[:, 0:128], rhs=warm, start=(wi == 0), stop=(wi == 47)
            )

        cp_tiles = {}

        def load_consts():
            # constants are only needed once projections produce results, so
            # their DMAs go AFTER the critical xk0/xq0 loads
            for name, ap, width in (
                ("mb_sb", mb, N_KT),
                ("bq_sb", bq, N_PAIRS),
                ("bk_sb", bk, N_PAIRS),
                ("bv_sb", bv, N_PAIRS * 130),
            ):
                t = cp.tile([128, width], F32, name=name)
                nc.sync.dma_start(t, ap)
                cp_tiles[name] = t

        # ---- streamed loads, priority-chunked: the first scores tile needs
        # only (wq, wk, xk chunk0, xq chunk0); the kt sweep then consumes the
        # rest of xk before more of xq is needed; v comes last. ----
        # Priority order: critical path to the first exp is wq+wk+xk0+xq0;
        # the kt sweep then needs the rest of xk at ~1.3us/kt; v (wv+xv)
        # must land before the deferred qc0 AV flush; the rest of xq is
        # needed one qc-block (~18us) at a time. Each load is ONE 1MB DMA
        # (dispatch on the sync engine costs ~650ns per DMA instruction, so
        # per-dt-tile DMAs would serialize on dispatch).
        def alloc_x(pfx):
            big = xp.tile([128, N_DT * SQ], BF16, name=f"{pfx}big", tag="x")
            return big, [big[:, i * SQ : (i + 1) * SQ] for i in range(N_DT)]

        def load_x_chunk(big, xap, c, eng=None):
            dst = big.rearrange("p (dt s) -> p dt s", dt=N_DT)[
                :, :, c * S_CHUNK : (c + 1) * S_CHUNK
            ]
            src = xap[c].rearrange("(dt p) s -> p dt s", p=128)
            (eng or nc.sync).dma_start(dst, src)

        def load_w(wap, pfx, width, eng=None):
            big = wp.tile([128, N_DT * width], BF16, name=f"{pfx}big", tag=pfx)
            (eng or nc.sync).dma_start(
                big.rearrange("p (dt w) -> p dt w", dt=N_DT),
                wap.rearrange("(dt p) w -> p dt w", p=128),
            )
            return [big[:, i * width : (i + 1) * width] for i in range(N_DT)]

        # All input DMAs go on ONE queue in strict priority order: HBM
        # bandwidth (~300 GB/s/core) is shared, so parallel queues just
        # steal bandwidth from the critical path. Land order = need order:
        # first exp needs wq+wk+xk0+xq0 (~18us); the kt sweep eats xk at
        # ~1.1us/kt; v (wv+xv chunks) must land by the qc0 AV flush (~40us);
        # xq chunks are needed one qc-block (~18us) apart.
        xq_big, xq_sb = alloc_x("xq")
        xk_big, xk_sb = alloc_x("xk")
        xv_big, xv_sb = alloc_x("xv")
        wq_sb = load_w(wq, "wq", HALF)
        wk_sb = load_w(wk, "wk", HALF)
        load_x_chunk(xk_big, xk, 0)
        load_x_chunk(xq_big, xq, 0)
        load_consts()
        mb_sb = cp_tiles["mb_sb"]
        bq_sb = cp_tiles["bq_sb"]
        bk_sb = cp_tiles["bk_sb"]
        bv_sb = cp_tiles["bv_sb"]
        wv_sb = load_w(wv, "wv", N_PAIRS * 130)
        load_x_chunk(xv_big, xv, 0)
        load_x_chunk(xk_big, xk, 1)
        load_x_chunk(xq_big, xq, 1)
        load_x_chunk(xk_big, xk, 2)
        load_x_chunk(xk_big, xk, 3)
        for c in range(1, N_QC):
            load_x_chunk(xv_big, xv, c)
        load_x_chunk(xq_big, xq, 2)
        load_x_chunk(xq_big, xq, 3)

        v_tiles = {}  # (pair, kt) -> [128, 130] bf16 tile
        qkT = {}  # (pfx, pair) -> [128, SQ] bf16 tile

        def qk_tile(pfx, pair):
            if (pfx, pair) not in qkT:
                qkT[(pfx, pair)] = qkvp.tile(
                    [128, SQ], BF16, name=f"{pfx}T{pair}", tag=f"{pfx}T", bufs=2
                )
            return qkT[(pfx, pair)]

        proj_ps_open = {}

        def emit_qk_chunk(pair, pfx, qc, half=None):
            # one [128, 512] projection chunk: 8 accumulating MMs + bias copy.
            # half=0/1 emits only the first/second 4 contraction MMs (filler
            # granularity); half=None emits the whole chunk.
            dst = qk_tile(pfx, pair)
            w_sb = wq_sb if pfx == "q" else wk_sb
            b_sb = bq_sb if pfx == "q" else bk_sb
            x_sb = xq_sb if pfx == "q" else xk_sb
            key = (pair, pfx, qc)
            if half == 1:
                ps = proj_ps_open.pop(key)
            else:
                ps = proj_ps.tile(
                    [128, S_CHUNK], F32, name=f"{pfx}ps{pair}_{qc}", tag="proj"
                )
            dts = range(N_DT) if half is None else range(half * 4, half * 4 + 4)
            for dt_i in dts:
                nc.tensor.matmul(
                    ps,
                    lhsT=w_sb[dt_i][:, pair * 128 : (pair + 1) * 128],
                    rhs=x_sb[dt_i][:, qc * S_CHUNK : (qc + 1) * S_CHUNK],
                    start=(dt_i == 0),
                    stop=(dt_i == N_DT - 1),
                )
            if half == 0:
                proj_ps_open[key] = ps
            else:
                nc.vector.tensor_scalar_add(
                    dst[:, qc * S_CHUNK : (qc + 1) * S_CHUNK],
                    ps,
                    b_sb[:, pair : pair + 1],
                )

        def emit_v_chunk(g, st):
            # v projection for pairs (2g, 2g+1), one key tile: N=260 matmuls
            ps = proj_ps.tile([128, S_CHUNK], F32, name=f"vps{g}_{st}", tag="proj")
            for dt_i in range(N_DT):
                nc.tensor.matmul(
                    ps[:, 0:260],
                    lhsT=xv_sb[dt_i][:, st * 128 : (st + 1) * 128],
                    rhs=wv_sb[dt_i][:, g * 260 : (g + 1) * 260],
                    start=(dt_i == 0),
                    stop=(dt_i == N_DT - 1),
                )
            for j in range(2):
                pair = 2 * g + j
                vt = qkvp.tile(
                    [128, 130], BF16, name=f"v{pair}_{st}", tag="v", bufs=3 * N_KT
                )
                nc.vector.tensor_add(
                    vt,
                    ps[:, j * 130 : (j + 1) * 130],
                    bv_sb[:, pair * 130 : (pair + 1) * 130],
                )
                v_tiles[(pair, st)] = vt

        # filler queue: projection chunk units (~0.85us of PE each), popped as
        # TensorE filler inside the attention stream. qk chunks are split into
        # 4-MM halves to match the per-slot budget. Ordering constraints:
        # qk(p) before pair-p attention, vg1 before pair-2 attention.
        filler = []

        def _qk_half(pair, pfx, qc, half):
            return lambda: emit_qk_chunk(pair, pfx, qc, half)

        for pfx in ("q", "k"):
            for c in range(N_QC):
                filler.append(_qk_half(1, pfx, c, 0))
                filler.append(_qk_half(1, pfx, c, 1))
        filler += [(lambda st=st: emit_v_chunk(1, st)) for st in range(N_KT)]
        for pfx in ("q", "k"):
            for c in range(N_QC):
                filler.append(_qk_half(2, pfx, c, 0))
                filler.append(_qk_half(2, pfx, c, 1))
        for pfx in ("q", "k"):
            for c in range(N_QC):
                filler.append(_qk_half(3, pfx, c, 0))
                filler.append(_qk_half(3, pfx, c, 1))

        def pop_filler():
            if filler:
                filler.pop(0)()

        # prologue: pair-0 projections, ordered so the first scores tile
        # (needing only the qc=0 chunks of qT0/kT0) unblocks ASAP
        emit_qk_chunk(0, "k", 0)
        emit_qk_chunk(0, "q", 0)
        for c in range(1, N_QC):
            emit_qk_chunk(0, "k", c)
        for c in range(1, N_QC):
            emit_qk_chunk(0, "q", c)
        for st in range(N_KT):
            emit_v_chunk(0, st)

        # ---- software-pipelined attention stream over (pair, qc, kt) ----
        iters = [
            (pair, qc, kt)
            for pair in range(N_PAIRS)
            for qc in range(N_QC)
            for kt in range(N_KT)
        ]
        sc_map = {}
        av_map = {}

        def emit_scores(i):
            pair, qc, kt = iters[i]
            qT = qk_tile("q", pair)
            kT = qk_tile("k", pair)
            sc = sc_ps.tile([128, 1024], F32, name=f"sc{pair}_{qc}_{kt}", tag="sc")
            # scoresT for heads A and B, packed in PE row groups
            nc.tensor.matmul(
                sc[:, 0:512],
                lhsT=kT[0:64, kt * 128 : (kt + 1) * 128],
                rhs=qT[0:64, qc * S_CHUNK : (qc + 1) * S_CHUNK],
                start=True,
                stop=True,
            )
            nc.tensor.matmul(
                sc[:, 512:1024],
                lhsT=kT[64:128, kt * 128 : (kt + 1) * 128],
                rhs=qT[64:128, qc * S_CHUNK : (qc + 1) * S_CHUNK],
                start=True,
                stop=True,
            )
            sc_map[i] = sc

        def emit_exp(i):
            # returns the bf16 [128, 1024] exp tile for iteration i
            pair, qc, kt = iters[i]
            sc = sc_map.pop(i)
            ex = expp.tile(
                [128, 1024], BF16, name=f"ex{pair}_{qc}_{kt}", tag="ex", bufs=20
            )
            nc.scalar.activation(
                ex,
                sc,
                mybir.ActivationFunctionType.Exp,
                bias=mb_sb[:, kt : kt + 1],
                scale=0.125,
            )
            return ex

        def emit_epilogue(pair, qc, av_a, av_b):
            # [65, 512] numerator+den blocks -> SBUF -> DRAM (host normalizes)
            stg = stgp.tile([65, 1024], F32, name=f"st{pair}_{qc}", tag="stg")
            nc.vector.tensor_copy(stg[:, 0:512], av_a)
            nc.vector.tensor_copy(stg[:, 512:1024], av_b)
            for h_i in range(2):
                nc.sync.dma_start(
                    out[
                        pair * 130 + h_i * 65 : pair * 130 + h_i * 65 + 65,
                        qc * S_CHUNK : (qc + 1) * S_CHUNK,
                    ],
                    stg[:, h_i * 512 : (h_i + 1) * 512],
                )

        def emit_av(pair, qc, kt, ex):
            if kt == 0:
                av_map[(pair, qc)] = (
                    av_ps.tile([65, S_CHUNK], F32, name=f"ava{pair}_{qc}", tag="av"),
                    av_ps.tile([65, S_CHUNK], F32, name=f"avb{pair}_{qc}", tag="av"),
                )
            av_a, av_b = av_map[(pair, qc)]
            nc.tensor.matmul(
                av_a,
                lhsT=v_tiles[(pair, kt)][:, 0:65],
                rhs=ex[:, 0:512],
                start=(kt == 0),
                stop=(kt == N_KT - 1),
            )
            nc.tensor.matmul(
                av_b,
                lhsT=v_tiles[(pair, kt)][:, 65:130],
                rhs=ex[:, 512:1024],
                start=(kt == 0),
                stop=(kt == N_KT - 1),
            )

        # Emission in 2-iteration blocks, software-pipelined:
        #   block b: exps (2b, 2b+1) | AV burst (2b-2, 2b-1) | scores (2b+2,
        #   2b+3) | one filler unit. The AV inputs are always two blocks old,
        #   so the 4-matmul AV burst never waits mid-stream; batching halves
        #   the PE stream-switch tax. Iters 0..15 (pair 0, qc 0) defer their
        #   AVs entirely so the exp engines start while the v projection still
        #   waits on the xv DMA.
        emit_scores(0)
        emit_scores(1)
        ex_map = {}
        n_it = len(iters)

        def emit_av_i(i):
            pair, qc, kt = iters[i]
            emit_av(pair, qc, kt, ex_map.pop(i))
            if kt == N_KT - 1:
                emit_epilogue(pair, qc, *av_map.pop((pair, qc)))

        next_av = 0
        for b in range(n_it // 2):
            i0, i1 = 2 * b, 2 * b + 1
            for i in (i0, i1):
                ex_map[i] = emit_exp(i)
            if b >= 8:
                # warmup: (pair 0, qc 0) AVs were deferred while xv was in
                # flight; catch up at <=6 AVs per block to keep the PE queue
                # from blocking on not-yet-projected v tiles. In the final
                # qc block, drain AVs with no lag to shorten the tail.
                target = 2 * b + 2 if b >= n_it // 2 - 8 else 2 * b - 2
                n_emit = min(6, target - next_av)
                for _ in range(n_emit):
                    emit_av_i(next_av)
                    next_av += 1
            if i1 + 2 < n_it:
                emit_scores(i1 + 1)
                emit_scores(i1 + 2)
            # spread the projection filler evenly (3 of every 4 blocks)
            # instead of front-loading it into pairs 0-1
            if b % 4 != 3:
                pop_filler()
        # drain the remaining AVs + epilogues
        while next_av < n_it:
            emit_av_i(next_av)
            next_av += 1

        assert not filler, f"{len(filler)} filler chunks left unscheduled"
        assert not ex_map and not av_map and not sc_map


def _prep_core_inputs(pre_qs, pre_ks, pre_vs, k_mask, q_w, q_b, k_w, k_b, v_w, v_b, core):
    b = core // 2
    hh = core % 2
    cols = slice(HALF * hh, HALF * (hh + 1))

    def chunk_blocked(x):
        # [S, D_PRE] -> [N_QC, D_PRE, S_CHUNK] contiguous blocks of x^T
        xt = x.T.astype(BF16_NP)  # [D_PRE, S]
        return np.ascontiguousarray(
            xt.reshape(D_PRE, N_QC, S_CHUNK).transpose(1, 0, 2)
        )

    xq = chunk_blocked(pre_qs[b])
    xk = chunk_blocked(pre_ks[b])
    xv = chunk_blocked(pre_vs[b])
    wq = np.ascontiguousarray(q_w[:, cols]).astype(BF16_NP)
    wk = np.ascontiguousarray(k_w[:, cols]).astype(BF16_NP)

    wv_core = v_w[:, cols].astype(np.float32)
    wv = np.zeros((D_PRE, N_PAIRS * 130), dtype=np.float32)
    bv_core = v_b[cols].astype(np.float32)
    bv_ext = np.zeros(N_PAIRS * 130, dtype=np.float32)
    for p in range(N_PAIRS):
        wv[:, p * 130 : p * 130 + 64] = wv_core[:, p * 128 : p * 128 + 64]
        wv[:, p * 130 + 65 : p * 130 + 129] = wv_core[:, p * 128 + 64 : p * 128 + 128]
        bv_ext[p * 130 : p * 130 + 64] = bv_core[p * 128 : p * 128 + 64]
        bv_ext[p * 130 + 64] = 1.0
        bv_ext[p * 130 + 65 : p * 130 + 129] = bv_core[p * 128 + 64 : p * 128 + 128]
        bv_ext[p * 130 + 129] = 1.0
    wv = wv.astype(BF16_NP)

    bq = np.ascontiguousarray(q_b[cols].astype(np.float32).reshape(N_PAIRS, 128).T)
    bk = np.ascontiguousarray(k_b[cols].astype(np.float32).reshape(N_PAIRS, 128).T)
    bv_full = np.ascontiguousarray(np.tile(bv_ext[None, :], (128, 1)))

    # ScalarE path: mask True -> -SHIFT, False -> MASK_NEG (exp underflows to 0)
    mrow = np.where(k_mask[b], -SHIFT, MASK_NEG).astype(np.float32)
    mb = np.ascontiguousarray(mrow.reshape(N_KT, 128).T)
    # VectorE fast-exp path: masked-out keys get A=0, B=MAGIC (-> bf16 bits 0)
    arow = np.where(k_mask[b], FE_A, 0.0).astype(np.float32)
    brow = np.where(k_mask[b], FE_B + MAGIC, MAGIC).astype(np.float32)
    dvea = np.ascontiguousarray(arow.reshape(N_KT, 128).T)
    dveb = np.ascontiguousarray(brow.reshape(N_KT, 128).T)

    return {
        "xq": xq,
        "xk": xk,
        "xv": xv,
        "wq": wq,
        "wk": wk,
        "wv": wv,
        "bq": bq,
        "bk": bk,
        "bv": bv_full,
        "mb": mb,
        "dvea": dvea,
        "dveb": dveb,
    }


def kernel(pre_qs, pre_ks, pre_vs, k_mask, q_w, q_b, k_w, k_b, v_w, v_b):
    global _COMPILED
    args = (pre_qs, pre_ks, pre_vs, k_mask, q_w, q_b, k_w, k_b, v_w, v_b)
    args = tuple(np.asarray(a) for a in args)

    if _COMPILED is None:
        _COMPILED = _build_program()
    nc = _COMPILED

    in_maps = [_prep_core_inputs(*args, core=c) for c in range(N_CORES)]

    trace = bool(int(os.environ.get("BASS_KERNEL_TRACE", "0")))
    res = run_bass_kernel_spmd(
        nc,
        in_maps,
        core_ids=list(range(N_CORES)),
        trace=trace,
    )
    if trace:
        kernel.last_results = res

    out = np.empty((B, SQ, H * D_V), dtype=np.float32)
    for c in range(N_CORES):
        b = c // 2
        hh = c % 2
        r = res.results[c]["out"]  # [520, 2048] fp32
        for p in range(N_PAIRS):
            for h_i in range(2):
                blk = r[p * 130 + h_i * 65 : p * 130 + h_i * 65 + 65]
                num = blk[0:64]
                den = blk[64]
                head = hh * 8 + 2 * p + h_i
                out[b, :, head * 64 : (head + 1) * 64] = (num / den).T
    return out
